# revision 56
# baseline (speedup 1.0000x reference)
"""Trainium2 Bass kernel for nn_DynamicHead — contiguous sharding + linear basis.

Within a knot segment the function is an exact cubic in t.  Sort all samples
by t, give each core a contiguous range of 4096, and split each core's range
into knot-pure chunks of <= 512 samples.  Each chunk spans a t-width of only
~0.016, so after recentering at the chunk midpoint a LINEAR basis [1, dt]
suffices (quadratic/cubic folded minimax-style into [1, dt], residual ~2e-4).

v2: the linear-term matmul runs in fp8 DoubleRow mode (z1 = x*dt' in e5m2,
W_l*8 in e4m3, K=256 per pass) — the term is ~1% of the output so fp8 noise
is invisible (lab: 9.55e-3 vs 9.52e-3 max-rel).  Bias weights ride inside the
main weight tile (partitions 0-1 of cols 512:767) so no slow 2-partition DMA.
Main term stays bf16: 2 k-tile matmuls per output half.  Per slot-layer-half:
2 bf16 + 1 DR + 1 bias matmul accumulate into one PSUM bank, ACT relu-evac.

Rings: vector = xin + t broadcasts, sync = L0 weights, gpsimd = head consts +
L1 weights, scalar = out stores (ACT_TABLE_LOAD blocks scalar early, so
nothing latency-critical sits there).  Small warmup matmuls open the PE
clock gate without serializing real work behind them.
"""
import os
import sys
import types

for _p in ('/opt/trn_rl_repo', '/root/.axon_site/_ro/trn_rl_repo'):
    if _p not in sys.path:
        sys.path.append(_p)

import numpy as np
import ml_dtypes
import concourse.bass as bass
import concourse.tile as tile
from concourse import bacc, mybir
from concourse import bass_utils

F32 = mybir.dt.float32
BF16 = mybir.dt.bfloat16
E4 = mybir.dt.float8e4
E5 = mybir.dt.float8e5
NPBF = ml_dtypes.bfloat16
NPE4 = ml_dtypes.float8_e4m3
RELU = mybir.ActivationFunctionType.Relu
COPY = mybir.ActivationFunctionType.Copy
DR = mybir.MatmulPerfMode.DoubleRow

B, D, NSEG = 32768, 256, 9
NP = 2                                  # linear centered basis [1, dt]
CAPMAX = 512
N_CORES = 8
BPC = B // N_CORES
KNOTS = np.array([i / 9.0 for i in range(1, 9)], dtype=np.float64)
SDIM = 12
ZSH = 3                                 # z1 = x*(dt*2^-ZSH) e5m2, W_l*2^ZSH e4m3
WU_N = int(os.environ.get("WU_N", "88"))
WU_C = int(os.environ.get("WU_C", "64"))

TRACE = False
LAST_EXEC_NS = None
LAST_MEAN_EXEC_NS = None
LAST_RES = None

_PROG_CACHE = {}

if os.environ.get("BASS_LDW_OPT") == "1":
    _orig_run_command = bass_utils.run_command

    def _run_command_ldw(argv, **kw):
        argv = ["--enable-ldw-opt=true" if a == "--enable-ldw-opt=false" else a
                for a in argv]
        return _orig_run_command(argv, **kw)

    bass_utils.run_command = _run_command_ldw


def _register_ntff_hook():
    try:
        import antenv.axon_hooks  # noqa: F401
        return
    except ImportError:
        pass
    try:
        from trn_agent_boot.trn_boot import _ntff_profile_via_ctypes
        hook = _ntff_profile_via_ctypes('/opt/axon/libaxon_pjrt.so')
        mod = types.ModuleType('antenv.axon_hooks')
        mod.get_axon_ntff_profile_hook = lambda: hook
        sys.modules['antenv.axon_hooks'] = mod
    except Exception:
        pass


def _gamma4() -> np.ndarray:
    """(NSEG, 4, SDIM): basis -> per-segment cubic coefficients (t-basis)."""
    g = np.zeros((NSEG, 4, SDIM), dtype=np.float64)
    for m in range(NSEG):
        for p in range(4):
            g[m, p, p] = 1.0
        for j in range(1, 9):          # spline s = 3 + j, knot k = j/9
            if j <= m:
                k = KNOTS[j - 1]
                g[m, 0, 3 + j] = -k ** 3
                g[m, 1, 3 + j] = 3 * k ** 2
                g[m, 2, 3 + j] = -3 * k
                g[m, 3, 3 + j] = 1.0
    return g


def _relin(c4, t0, h):
    """cubic coeffs (4, ...) in t-basis -> linear (2, ...) in dt-basis.

    Taylor recenter at t0, then Chebyshev minimax folds on [-h, h]:
    dt^2 ~ h^2/2 (into const), dt^3 ~ (3h^2/4) dt (into linear)."""
    from math import comb
    c = np.zeros((4,) + c4.shape[1:])
    for q in range(4):
        for p in range(q, 4):
            c[q] += comb(p, q) * (t0 ** (p - q)) * c4[p]
    out = c[:2].copy()
    out[0] += 0.5 * h * h * c[2]
    out[1] += 0.75 * h * h * c[3]
    return out


def _build_program(caps):
    """SPMD single-core program: NSLOT chunks with per-slot capacities."""
    caps = tuple(int(c) for c in caps)
    nslot = len(caps)
    offs = [0]
    for c in caps:
        offs.append(offs[-1] + c)
    bp = offs[-1]
    nc = bacc.Bacc("TRN2", target_bir_lowering=False, debug=False,
                   num_devices=N_CORES)

    xT_ap = nc.dram_tensor("xT", [128, 2 * bp], BF16, kind="ExternalInput").ap()
    tp_ap = nc.dram_tensor("tp", [3, bp], BF16, kind="ExternalInput").ap()
    # merged weight tensor per slot-layer: [0:512] bf16 main,
    # [512:768] = 512 e4m3 linear-term bytes packed as 256 bf16 cols
    cm0_ap = nc.dram_tensor("cm0", [nslot, 128, 768], BF16, kind="ExternalInput").ap()
    cm1_ap = nc.dram_tensor("cm1", [nslot, 128, 768], BF16, kind="ExternalInput").ap()
    # bias weights for all slot-layers in one small 2-partition tensor
    cb_ap = nc.dram_tensor("cball", [NP, nslot * 512], BF16, kind="ExternalInput").ap()
    c2_ap = nc.dram_tensor("c2", [128, nslot * 2 * NP], BF16, kind="ExternalInput").ap()
    c2b_ap = nc.dram_tensor("c2b", [NP, nslot], F32, kind="ExternalInput").ap()
    # out rows (q0+b0) and (q1+b1)*dt stored separately; host adds them
    out_ap = nc.dram_tensor("out", [NP, bp], BF16, kind="ExternalOutput").ap()

    cm_ap = (cm0_ap, cm1_ap)
    imap = {}

    def tag_inst(inst, label):
        try:
            imap[inst.ins.name] = label
        except Exception:
            pass

    with tile.TileContext(nc) as tc:
        with (
            tc.tile_pool(name="act", bufs=1) as actp,
            tc.tile_pool(name="z", bufs=1) as zp,
            tc.tile_pool(name="w", bufs=1) as wp,
            tc.tile_pool(name="sm", bufs=1) as smp,
            tc.tile_pool(name="pm", bufs=1, space="PSUM") as pmp,
            tc.tile_pool(name="pq", bufs=1, space="PSUM") as pqp,
        ):
            # ---- warmup: many small matmuls open the PE clock gate during
            # the DMA prologue without serializing real work behind them.
            wu = smp.tile([128, max(WU_C, 128)], BF16, name="wu", tag="wu")
            nc.vector.memset(wu[:, :], 0)
            pwu = pqp.tile([128, 512], F32, name="pwu", tag="pq", bufs=2)
            for _ in range(WU_N):
                nc.tensor.matmul(pwu[:, 0:WU_C], wu[:, 0:128], wu[:, 0:WU_C],
                                 start=True, stop=True)

            wts = {}

            def wload(L, s):
                # L0 weights on the sync ring, L1 on the gpsimd ring: two
                # HWDGE rings in parallel so weight supply keeps up with PE.
                if (L, s) in wts:
                    return
                wm = wp.tile([128, 768], BF16, name=f"wm{L}_{s}",
                             tag=f"wm{L}", bufs=4)
                eng = nc.sync if L == 0 else nc.gpsimd
                tag_inst(eng.dma_start(wm[:, :], cm_ap[L][s]), f"dma:w{L}:{s}")
                wts[(L, s)] = wm

            # ---- per-slot input loads: xin on the sync ring (interleaved
            # with L0 weights), dt-broadcast on the gpsimd ring.  tps is one
            # 2-partition load on the scalar ring (slow line, early issue,
            # first needed only by slot0's 4th matmul).  scalar otherwise
            # only carries out stores (ACT_TABLE_LOAD blocks it until ~8us).
            xin, x1, x2, t1b = {}, {}, {}, {}

            def load_seg(s):
                cap, off = caps[s], offs[s]
                xt = actp.tile([128, 2 * cap], BF16, name=f"xin{s}",
                               tag="xin", bufs=4)
                # xin0 on the fast-starting sync ring (scalar is blocked
                # early, gpsimd starts late); the rest ride the scalar ring
                eng = nc.sync if s < 1 else nc.scalar
                tag_inst(eng.dma_start(xt[:, :],
                                       xT_ap[:, 2 * off:2 * off + 2 * cap]),
                         f"dma:xin:{s}")
                xin[s] = xt
                tb = smp.tile([128, cap], BF16, name=f"t1_{s}",
                              tag="t1", bufs=4)
                tag_inst(nc.gpsimd.dma_start(
                    tb[:, :], tp_ap[2:3, off:off + cap].partition_broadcast(128)),
                    f"dma:t1b:{s}")
                t1b[s] = tb

            load_seg(0)
            wload(0, 0)
            # tiny slot0 bias/tps slices ride the fast sync ring right after
            # w00 so slot0's groups can close early; the slow full
            # 2-partition tensors follow on the gpsimd ring
            cap0 = caps[0]
            cbt0 = smp.tile([NP, 512], BF16, name="cbt0", tag="cbt0")
            tag_inst(nc.sync.dma_start(cbt0[:, :], cb_ap[:, 0:512]),
                     "dma:cb0")
            tps0 = smp.tile([NP, cap0], BF16, name="tps0", tag="tps0")
            tag_inst(nc.sync.dma_start(tps0[:, :], tp_ap[0:NP, 0:cap0]),
                     "dma:tps0")
            wload(1, 0)
            tps = smp.tile([NP, bp], BF16, name="tps", tag="tps")
            tag_inst(nc.gpsimd.dma_start(tps[:, :], tp_ap[0:NP, :]), "dma:tps")
            cbt = smp.tile([NP, nslot * 512], BF16, name="cbt", tag="cbt")
            tag_inst(nc.gpsimd.dma_start(cbt[:, :], cb_ap[:, :]), "dma:cb")
            load_seg(1)
            wload(0, 1)
            wload(0, 2)

            # head consts on the gpsimd ring (tiny; needed from step 2)
            c2t = smp.tile([128, nslot * 2 * NP], BF16, name="c2t", tag="c2t")
            nc.gpsimd.dma_start(c2t[:, :], c2_ap[:, :])
            c2b = smp.tile([NP, nslot], F32, name="c2b", tag="c2b")
            nc.gpsimd.dma_start(c2b[:, :], c2b_ap[:, :])
            wload(1, 1)
            load_seg(2)
            load_seg(3)

            def vc_layer(s, L, xin_t, store):
                """layers 0/1: (o,b) = relu(Wc.T@x + 8Wl.T@z1 + cb.T@tps)"""
                cap, off = caps[s], offs[s]
                if (L, s) not in wts:
                    wload(L, s)
                wm = wts.pop((L, s))

                z1 = zp.tile([128, 2 * cap], E5, name=f"z1_{L}_{s}",
                             tag="z1", bufs=3)
                # fp8 writes are slow (1 col/cyc DVE, worse on Pool):
                # 3 of 4 muls on DVE, L1-h1 on Pool
                for h in range(2):
                    zeng = nc.gpsimd if (L == 1 and h == 1) else nc.vector
                    tag_inst(zeng.tensor_mul(z1[:, h * cap:(h + 1) * cap],
                                             xin_t[:, h * cap:(h + 1) * cap],
                                             t1b[s][:, :]), f"z1:{L}:{s}:{h}")
                z13 = z1[:, :].rearrange("p (j c) -> p j c", j=2)
                wl3 = wm[:, 512:768].bitcast(E4).rearrange(
                    "p (j c) -> p j c", j=2)
                xo = actp.tile([128, 2 * cap], BF16, name=f"x{L + 1}_{s}",
                               tag=f"xo{L}", bufs=3)
                # interleave the two output halves' accumulation groups so
                # each group's open/close latency drains under the other
                # group's column streaming (alternating PSUM banks)
                pss = [pmp.tile([128, cap], F32, name=f"pm{L}_{s}_{m}",
                                tag="pm", bufs=6) for m in range(2)]
                for m in range(2):
                    tag_inst(nc.tensor.matmul(
                        pss[m][:, :], wm[:, (m * 2) * 128:(m * 2 + 1) * 128],
                        xin_t[:, 0:cap],
                        start=True, stop=False), f"mm:{L}:{s}:{m}:kt0")
                for m in range(2):
                    tag_inst(nc.tensor.matmul(
                        pss[m][:, :],
                        wm[:, (m * 2 + 1) * 128:(m * 2 + 2) * 128],
                        xin_t[:, cap:2 * cap],
                        start=False, stop=False), f"mm:{L}:{s}:{m}:kt1")
                for m in range(2):
                    tag_inst(nc.tensor.matmul(
                        pss[m][:, :], wl3[:, :, m * 128:(m + 1) * 128], z13,
                        start=False, stop=False, perf_mode=DR),
                        f"mm:{L}:{s}:{m}:DR")
                cb_t = cbt0 if s == 0 else cbt
                cb_o = L * 256 if s == 0 else (s * 2 + L) * 256
                tp_t = tps0[0:NP, :] if s == 0 else tps[0:NP, off:off + cap]
                for m in range(2):
                    tag_inst(nc.tensor.matmul(
                        pss[m][:, :],
                        cb_t[0:NP, cb_o + m * 128:cb_o + (m + 1) * 128],
                        tp_t,
                        start=False, stop=True), f"mm:{L}:{s}:{m}:bias")
                    tag_inst(nc.scalar.activation(
                        xo[:, m * cap:(m + 1) * cap], pss[m][:, :], RELU),
                        f"act:{L}:{s}:{m}")
                store[s] = xo

            def head_layer(s):
                """layer 2 (out_dim=1): q = C2.T @ x2; out = (q0+b0) + (q1+b1)*dt"""
                cap, off = caps[s], offs[s]
                psq = pqp.tile([NP, cap], F32, name=f"pq{s}", tag="pq", bufs=2)
                for h in range(2):
                    tag_inst(nc.tensor.matmul(
                        psq[:, :],
                        c2t[:, s * 2 * NP + h * NP:s * 2 * NP + (h + 1) * NP],
                        x2[s][:, h * cap:(h + 1) * cap],
                        start=(h == 0), stop=(h == 1)), f"mm:h:{s}:{h}:q")
                # fused (q + b) * tps in one DVE op; rows summed on HOST
                rq = smp.tile([NP, cap], BF16, name=f"rq{s}", tag="rq", bufs=3)
                tag_inst(nc.vector.scalar_tensor_tensor(
                    rq[:, :], psq[:, :], c2b[0:NP, s:s + 1],
                    tps0[0:NP, :] if s == 0 else tps[0:NP, off:off + cap],
                    mybir.AluOpType.add, mybir.AluOpType.mult), f"stst:{s}")
                tag_inst(nc.scalar.dma_start(out_ap[0:NP, off:off + cap],
                                             rq[:, :]), f"dma:out:{s}")

            for step in range(nslot + 2):
                # prefetch L1 weights so their ring issues precede the Pool
                # z1 ops in queue order (Pool queue is in-order; z1 waits
                # on the L0 evac and would stall the issue otherwise)
                if step + 1 < nslot:
                    wload(1, step + 1)
                if step < nslot:
                    if step not in xin:
                        load_seg(step)
                    vc_layer(step, 0, xin[step], x1)
                    xin.pop(step)
                if 1 <= step < nslot + 1:
                    vc_layer(step - 1, 1, x1[step - 1], x2)
                    x1.pop(step - 1)
                if step >= 2:
                    head_layer(step - 2)
                    x2.pop(step - 2)
                    t1b.pop(step - 2)

    nc.compile()
    import json
    with open('/tmp/imap.json', 'w') as f:
        json.dump(imap, f)
    return nc


def _prep_host(treatment, features, W0, b0, W1, b1, W2, b2):
    t32 = np.asarray(treatment, dtype=np.float32)
    t = t32.astype(np.float64)
    x = np.asarray(features, dtype=np.float32)

    order = np.argsort(t32, kind='stable')
    percore = order.reshape(N_CORES, BPC)
    kn32 = KNOTS.astype(np.float32)

    chunk_lists = []                        # per core: list of index arrays
    for c in range(N_CORES):
        gi = percore[c]
        tc_ = t32[gi]
        bounds = {0, len(gi)}
        for k in kn32:
            bounds.add(int(np.searchsorted(tc_, k, side='right')))
        bounds = sorted(bounds)
        chunks = []
        for a, b_ in zip(bounds, bounds[1:]):
            r = b_ - a
            if r <= 0:
                continue
            k = -(-r // CAPMAX)
            base, rem = divmod(r, k)
            st = a
            for i in range(k):
                n = base + (1 if i < rem else 0)
                chunks.append(gi[st:st + n])
                st += n
        chunks.sort(key=len, reverse=True)
        chunk_lists.append(chunks)

    nslot = max(len(ch) for ch in chunk_lists)
    caps = tuple(max(8, -(-max(len(ch[i]) if i < len(ch) else 0
                               for ch in chunk_lists) // 8) * 8)
                 for i in range(nslot))
    assert max(caps) <= 512, caps
    offs = np.concatenate([[0], np.cumsum(caps)]).astype(np.int64)
    bp = int(offs[-1])

    g = _gamma4()
    c4s, cb4s = [], []
    for W, b in ((W0, b0), (W1, b1)):
        Ws = np.asarray(W, dtype=np.float64).reshape(SDIM, D, D)
        c4s.append(np.einsum('mps,sio->mpio', g, Ws))
        cb4s.append(np.einsum('mps,so->mpo', g, np.asarray(b, np.float64)))
    c4h = np.einsum('mps,si->mpi', g, np.asarray(W2, np.float64))
    cb4h = np.einsum('mps,s->mp', g, np.asarray(b2, np.float64)[:, 0])

    gather = np.full((N_CORES, bp), -1, dtype=np.int64)
    in_maps = []
    for c in range(N_CORES):
        chunks = chunk_lists[c]
        xT = np.zeros((128, 2 * bp), dtype=NPBF)
        tp = np.zeros((3, bp), dtype=NPBF)
        cm = [np.zeros((nslot, 128, 768), dtype=NPBF) for _ in range(2)]
        cball = np.zeros((NP, nslot * 512), dtype=NPBF)
        c2 = np.zeros((128, nslot * 2 * NP), dtype=NPBF)
        c2b = np.zeros((NP, nslot), dtype=np.float32)
        for s, gi in enumerate(chunks):
            n, off = len(gi), int(offs[s])
            tv = t[gi]
            t0 = (tv.min() + tv.max()) / 2
            h = max((tv.max() - tv.min()) / 2, 1e-9)
            m = int(np.searchsorted(kn32, t32[gi[0]], side='right'))
            gather[c, off:off + n] = gi
            cap_s = caps[s]
            xT[:, 2 * off:2 * off + n] = x[gi, 0:128].T.astype(NPBF)
            xT[:, 2 * off + cap_s:2 * off + cap_s + n] = \
                x[gi, 128:256].T.astype(NPBF)
            dv = tv - t0
            tp[0, off:off + n] = 1.0
            tp[1, off:off + n] = dv.astype(NPBF)
            tp[2, off:off + n] = (dv * 2.0 ** -ZSH).astype(NPBF)
            for L in range(2):
                c2l = _relin(c4s[L][m], t0, h)              # (2, 256, 256)
                wc, wl8 = c2l[0], c2l[1] * 2.0 ** ZSH
                for mm in range(2):
                    for kt in range(2):
                        cm[L][s, :, (mm * 2 + kt) * 128:(mm * 2 + kt + 1) * 128] = \
                            wc[kt * 128:(kt + 1) * 128,
                               mm * 128:(mm + 1) * 128].astype(NPBF)
                cbl = _relin(cb4s[L][m], t0, h)             # (2, 256)
                cball[:, (s * 2 + L) * 256:(s * 2 + L + 1) * 256] = \
                    cbl.astype(NPBF)
                # fp8 linear weights packed as raw bytes into bf16 cols
                wl_q = np.zeros((128, 512), dtype=NPE4)
                for j in range(2):
                    wl_q[:, j * 256:(j + 1) * 256] = \
                        wl8[j * 128:(j + 1) * 128, :].astype(NPE4)
                cm[L][s, :, 512:768] = \
                    np.ascontiguousarray(wl_q).view(np.uint8).reshape(
                        128, 512).view(NPBF)
            c2h = _relin(c4h[m], t0, h)                     # (2, 256)
            for hh in range(2):
                c2[:, s * 2 * NP + hh * NP:s * 2 * NP + (hh + 1) * NP] = \
                    c2h[:, hh * 128:(hh + 1) * 128].T.astype(NPBF)
            c2b[:, s] = _relin(cb4h[m][:, None], t0, h)[:, 0].astype(np.float32)
        in_maps.append(dict(
            xT=np.ascontiguousarray(xT), tp=np.ascontiguousarray(tp),
            cm0=np.ascontiguousarray(cm[0]), cm1=np.ascontiguousarray(cm[1]),
            cball=cball, c2=c2, c2b=c2b))
    return caps, in_maps, gather


def kernel(treatment, features, W0, b0, W1, b1, W2, b2):
    global LAST_EXEC_NS, LAST_MEAN_EXEC_NS, LAST_RES
    caps, in_maps, gather = _prep_host(treatment, features, W0, b0, W1, b1,
                                       W2, b2)

    if caps not in _PROG_CACHE:
        _PROG_CACHE[caps] = _build_program(caps)
    nc = _PROG_CACHE[caps]

    if TRACE:
        _register_ntff_hook()
    res = bass_utils.run_bass_kernel_spmd(
        nc, in_maps, core_ids=list(range(N_CORES)), trace=TRACE)
    LAST_EXEC_NS = res.exec_time_ns
    LAST_MEAN_EXEC_NS = res.mean_exec_time_ns
    LAST_RES = res

    out = np.empty((B,), dtype=np.float32)
    for c in range(N_CORES):
        row = res.results[c]["out"]      # (2, bp) bf16: host sums the rows
        v = gather[c] >= 0
        s = row[0].astype(np.float32) + row[1].astype(np.float32)
        out[gather[c][v]] = s[v]
    return out.reshape(B, 1)


# revision 57
# speedup vs baseline: 1.0302x; 1.0302x over previous
"""Trainium2 Bass kernel for nn_DynamicHead — contiguous sharding + linear basis.

Within a knot segment the function is an exact cubic in t.  Sort all samples
by t, give each core a contiguous range of 4096, and split each core's range
into knot-pure chunks of <= 512 samples.  Each chunk spans a t-width of only
~0.016, so after recentering at the chunk midpoint a LINEAR basis [1, dt]
suffices (quadratic/cubic folded minimax-style into [1, dt], residual ~2e-4).

v2: the linear-term matmul runs in fp8 DoubleRow mode (z1 = x*dt' in e5m2,
W_l*8 in e4m3, K=256 per pass) — the term is ~1% of the output so fp8 noise
is invisible (lab: 9.55e-3 vs 9.52e-3 max-rel).  Bias weights ride inside the
main weight tile (partitions 0-1 of cols 512:767) so no slow 2-partition DMA.
Main term stays bf16: 2 k-tile matmuls per output half.  Per slot-layer-half:
2 bf16 + 1 DR + 1 bias matmul accumulate into one PSUM bank, ACT relu-evac.

Rings: vector = xin + t broadcasts, sync = L0 weights, gpsimd = head consts +
L1 weights, scalar = out stores (ACT_TABLE_LOAD blocks scalar early, so
nothing latency-critical sits there).  Small warmup matmuls open the PE
clock gate without serializing real work behind them.
"""
import os
import sys
import types

for _p in ('/opt/trn_rl_repo', '/root/.axon_site/_ro/trn_rl_repo'):
    if _p not in sys.path:
        sys.path.append(_p)

import numpy as np
import ml_dtypes
import concourse.bass as bass
import concourse.tile as tile
from concourse import bacc, mybir
from concourse import bass_utils

F32 = mybir.dt.float32
BF16 = mybir.dt.bfloat16
E4 = mybir.dt.float8e4
E5 = mybir.dt.float8e5
NPBF = ml_dtypes.bfloat16
NPE4 = ml_dtypes.float8_e4m3
RELU = mybir.ActivationFunctionType.Relu
COPY = mybir.ActivationFunctionType.Copy
DR = mybir.MatmulPerfMode.DoubleRow

B, D, NSEG = 32768, 256, 9
NP = 2                                  # linear centered basis [1, dt]
CAPMAX = 512
N_CORES = 8
BPC = B // N_CORES
KNOTS = np.array([i / 9.0 for i in range(1, 9)], dtype=np.float64)
SDIM = 12
ZSH = 3                                 # z1 = x*(dt*2^-ZSH) e5m2, W_l*2^ZSH e4m3
WU_N = int(os.environ.get("WU_N", "88"))
WU_C = int(os.environ.get("WU_C", "64"))

TRACE = False
LAST_EXEC_NS = None
LAST_MEAN_EXEC_NS = None
LAST_RES = None

_PROG_CACHE = {}

if os.environ.get("BASS_LDW_OPT") == "1":
    _orig_run_command = bass_utils.run_command

    def _run_command_ldw(argv, **kw):
        argv = ["--enable-ldw-opt=true" if a == "--enable-ldw-opt=false" else a
                for a in argv]
        return _orig_run_command(argv, **kw)

    bass_utils.run_command = _run_command_ldw


def _register_ntff_hook():
    try:
        import antenv.axon_hooks  # noqa: F401
        return
    except ImportError:
        pass
    try:
        from trn_agent_boot.trn_boot import _ntff_profile_via_ctypes
        hook = _ntff_profile_via_ctypes('/opt/axon/libaxon_pjrt.so')
        mod = types.ModuleType('antenv.axon_hooks')
        mod.get_axon_ntff_profile_hook = lambda: hook
        sys.modules['antenv.axon_hooks'] = mod
    except Exception:
        pass


def _gamma4() -> np.ndarray:
    """(NSEG, 4, SDIM): basis -> per-segment cubic coefficients (t-basis)."""
    g = np.zeros((NSEG, 4, SDIM), dtype=np.float64)
    for m in range(NSEG):
        for p in range(4):
            g[m, p, p] = 1.0
        for j in range(1, 9):          # spline s = 3 + j, knot k = j/9
            if j <= m:
                k = KNOTS[j - 1]
                g[m, 0, 3 + j] = -k ** 3
                g[m, 1, 3 + j] = 3 * k ** 2
                g[m, 2, 3 + j] = -3 * k
                g[m, 3, 3 + j] = 1.0
    return g


def _relin(c4, t0, h):
    """cubic coeffs (4, ...) in t-basis -> linear (2, ...) in dt-basis.

    Taylor recenter at t0, then Chebyshev minimax folds on [-h, h]:
    dt^2 ~ h^2/2 (into const), dt^3 ~ (3h^2/4) dt (into linear)."""
    from math import comb
    c = np.zeros((4,) + c4.shape[1:])
    for q in range(4):
        for p in range(q, 4):
            c[q] += comb(p, q) * (t0 ** (p - q)) * c4[p]
    out = c[:2].copy()
    out[0] += 0.5 * h * h * c[2]
    out[1] += 0.75 * h * h * c[3]
    return out


def _build_program(caps):
    """SPMD single-core program: NSLOT chunks with per-slot capacities."""
    caps = tuple(int(c) for c in caps)
    nslot = len(caps)
    offs = [0]
    for c in caps:
        offs.append(offs[-1] + c)
    bp = offs[-1]
    nc = bacc.Bacc("TRN2", target_bir_lowering=False, debug=False,
                   num_devices=N_CORES)

    xT_ap = nc.dram_tensor("xT", [128, 2 * bp], BF16, kind="ExternalInput").ap()
    tp_ap = nc.dram_tensor("tp", [3, bp], BF16, kind="ExternalInput").ap()
    # merged weight tensor per slot-layer: [0:512] bf16 main,
    # [512:768] = 512 e4m3 linear-term bytes packed as 256 bf16 cols
    cm0_ap = nc.dram_tensor("cm0", [nslot, 128, 768], BF16, kind="ExternalInput").ap()
    cm1_ap = nc.dram_tensor("cm1", [nslot, 128, 768], BF16, kind="ExternalInput").ap()
    # bias weights for all slot-layers in one small 2-partition tensor
    cb_ap = nc.dram_tensor("cball", [NP, nslot * 512], BF16, kind="ExternalInput").ap()
    c2_ap = nc.dram_tensor("c2", [128, nslot * 2 * NP], BF16, kind="ExternalInput").ap()
    c2b_ap = nc.dram_tensor("c2b", [NP, nslot], F32, kind="ExternalInput").ap()
    # out rows (q0+b0) and (q1+b1)*dt stored separately; host adds them
    out_ap = nc.dram_tensor("out", [NP, bp], BF16, kind="ExternalOutput").ap()

    cm_ap = (cm0_ap, cm1_ap)
    imap = {}

    def tag_inst(inst, label):
        try:
            imap[inst.ins.name] = label
        except Exception:
            pass

    with tile.TileContext(nc) as tc:
        with (
            tc.tile_pool(name="act", bufs=1) as actp,
            tc.tile_pool(name="z", bufs=1) as zp,
            tc.tile_pool(name="w", bufs=1) as wp,
            tc.tile_pool(name="sm", bufs=1) as smp,
            tc.tile_pool(name="pm", bufs=1, space="PSUM") as pmp,
            tc.tile_pool(name="pq", bufs=1, space="PSUM") as pqp,
        ):
            # ---- warmup: many small matmuls open the PE clock gate during
            # the DMA prologue without serializing real work behind them.
            wu = smp.tile([128, max(WU_C, 128)], BF16, name="wu", tag="wu")
            nc.vector.memset(wu[:, :], 0)
            pwu = pqp.tile([128, 512], F32, name="pwu", tag="pq", bufs=2)
            for _ in range(WU_N):
                nc.tensor.matmul(pwu[:, 0:WU_C], wu[:, 0:128], wu[:, 0:WU_C],
                                 start=True, stop=True)

            wts = {}

            def wload(L, s):
                # L0 weights on the sync ring, L1 on the gpsimd ring: two
                # HWDGE rings in parallel so weight supply keeps up with PE.
                if (L, s) in wts:
                    return
                wm = wp.tile([128, 768], BF16, name=f"wm{L}_{s}",
                             tag=f"wm{L}", bufs=5)
                eng = nc.sync if L == 0 else nc.gpsimd
                tag_inst(eng.dma_start(wm[:, :], cm_ap[L][s]), f"dma:w{L}:{s}")
                wts[(L, s)] = wm

            # ---- per-slot input loads: xin on the sync ring (interleaved
            # with L0 weights), dt-broadcast on the gpsimd ring.  tps is one
            # 2-partition load on the scalar ring (slow line, early issue,
            # first needed only by slot0's 4th matmul).  scalar otherwise
            # only carries out stores (ACT_TABLE_LOAD blocks it until ~8us).
            xin, x1, x2, t1b = {}, {}, {}, {}

            def load_seg(s):
                cap, off = caps[s], offs[s]
                xt = actp.tile([128, 2 * cap], BF16, name=f"xin{s}",
                               tag="xin", bufs=5)
                # xin0 on the fast-starting sync ring (scalar is blocked
                # early, gpsimd starts late); the rest ride the scalar ring
                eng = nc.sync if s < 1 else nc.scalar
                tag_inst(eng.dma_start(xt[:, :],
                                       xT_ap[:, 2 * off:2 * off + 2 * cap]),
                         f"dma:xin:{s}")
                xin[s] = xt
                tb = smp.tile([128, cap], BF16, name=f"t1_{s}",
                              tag="t1", bufs=4)
                tag_inst(nc.gpsimd.dma_start(
                    tb[:, :], tp_ap[2:3, off:off + cap].partition_broadcast(128)),
                    f"dma:t1b:{s}")
                t1b[s] = tb

            load_seg(0)
            wload(0, 0)
            # tiny slot0 bias/tps slices ride the fast sync ring right after
            # w00 so slot0's groups can close early; the slow full
            # 2-partition tensors follow on the gpsimd ring
            cap0 = caps[0]
            cbt0 = smp.tile([NP, 512], BF16, name="cbt0", tag="cbt0")
            tag_inst(nc.sync.dma_start(cbt0[:, :], cb_ap[:, 0:512]),
                     "dma:cb0")
            tps0 = smp.tile([NP, cap0], BF16, name="tps0", tag="tps0")
            tag_inst(nc.sync.dma_start(tps0[:, :], tp_ap[0:NP, 0:cap0]),
                     "dma:tps0")
            wload(1, 0)
            tps = smp.tile([NP, bp], BF16, name="tps", tag="tps")
            tag_inst(nc.gpsimd.dma_start(tps[:, :], tp_ap[0:NP, :]), "dma:tps")
            cbt = smp.tile([NP, nslot * 512], BF16, name="cbt", tag="cbt")
            tag_inst(nc.gpsimd.dma_start(cbt[:, :], cb_ap[:, :]), "dma:cb")
            load_seg(1)
            wload(0, 1)
            wload(0, 2)

            # head consts on the gpsimd ring (tiny; needed from step 2)
            c2t = smp.tile([128, nslot * 2 * NP], BF16, name="c2t", tag="c2t")
            nc.gpsimd.dma_start(c2t[:, :], c2_ap[:, :])
            c2b = smp.tile([NP, nslot], F32, name="c2b", tag="c2b")
            nc.gpsimd.dma_start(c2b[:, :], c2b_ap[:, :])
            wload(1, 1)
            load_seg(2)
            load_seg(3)

            def vc_layer(s, L, xin_t, store):
                """layers 0/1: (o,b) = relu(Wc.T@x + 8Wl.T@z1 + cb.T@tps)"""
                cap, off = caps[s], offs[s]
                if (L, s) not in wts:
                    wload(L, s)
                wm = wts.pop((L, s))

                z1 = zp.tile([128, 2 * cap], E5, name=f"z1_{L}_{s}",
                             tag="z1", bufs=3)
                # fp8 writes are slow (1 col/cyc DVE, worse on Pool):
                # 3 of 4 muls on DVE, L1-h1 on Pool
                for h in range(2):
                    zeng = nc.gpsimd if (L == 1 and h == 1) else nc.vector
                    tag_inst(zeng.tensor_mul(z1[:, h * cap:(h + 1) * cap],
                                             xin_t[:, h * cap:(h + 1) * cap],
                                             t1b[s][:, :]), f"z1:{L}:{s}:{h}")
                z13 = z1[:, :].rearrange("p (j c) -> p j c", j=2)
                wl3 = wm[:, 512:768].bitcast(E4).rearrange(
                    "p (j c) -> p j c", j=2)
                xo = actp.tile([128, 2 * cap], BF16, name=f"x{L + 1}_{s}",
                               tag=f"xo{L}", bufs=3)
                # interleave the two output halves' accumulation groups so
                # each group's open/close latency drains under the other
                # group's column streaming (alternating PSUM banks)
                pss = [pmp.tile([128, cap], F32, name=f"pm{L}_{s}_{m}",
                                tag="pm", bufs=6) for m in range(2)]
                for m in range(2):
                    tag_inst(nc.tensor.matmul(
                        pss[m][:, :], wm[:, (m * 2) * 128:(m * 2 + 1) * 128],
                        xin_t[:, 0:cap],
                        start=True, stop=False), f"mm:{L}:{s}:{m}:kt0")
                for m in range(2):
                    tag_inst(nc.tensor.matmul(
                        pss[m][:, :],
                        wm[:, (m * 2 + 1) * 128:(m * 2 + 2) * 128],
                        xin_t[:, cap:2 * cap],
                        start=False, stop=False), f"mm:{L}:{s}:{m}:kt1")
                for m in range(2):
                    tag_inst(nc.tensor.matmul(
                        pss[m][:, :], wl3[:, :, m * 128:(m + 1) * 128], z13,
                        start=False, stop=False, perf_mode=DR),
                        f"mm:{L}:{s}:{m}:DR")
                cb_t = cbt0 if s == 0 else cbt
                cb_o = L * 256 if s == 0 else (s * 2 + L) * 256
                tp_t = tps0[0:NP, :] if s == 0 else tps[0:NP, off:off + cap]
                for m in range(2):
                    tag_inst(nc.tensor.matmul(
                        pss[m][:, :],
                        cb_t[0:NP, cb_o + m * 128:cb_o + (m + 1) * 128],
                        tp_t,
                        start=False, stop=True), f"mm:{L}:{s}:{m}:bias")
                    tag_inst(nc.scalar.activation(
                        xo[:, m * cap:(m + 1) * cap], pss[m][:, :], RELU),
                        f"act:{L}:{s}:{m}")
                store[s] = xo

            def head_layer(s):
                """layer 2 (out_dim=1): q = C2.T @ x2; out = (q0+b0) + (q1+b1)*dt"""
                cap, off = caps[s], offs[s]
                psq = pqp.tile([NP, cap], F32, name=f"pq{s}", tag="pq", bufs=2)
                for h in range(2):
                    tag_inst(nc.tensor.matmul(
                        psq[:, :],
                        c2t[:, s * 2 * NP + h * NP:s * 2 * NP + (h + 1) * NP],
                        x2[s][:, h * cap:(h + 1) * cap],
                        start=(h == 0), stop=(h == 1)), f"mm:h:{s}:{h}:q")
                # fused (q + b) * tps in one DVE op; rows summed on HOST
                rq = smp.tile([NP, cap], BF16, name=f"rq{s}", tag="rq", bufs=3)
                tag_inst(nc.vector.scalar_tensor_tensor(
                    rq[:, :], psq[:, :], c2b[0:NP, s:s + 1],
                    tps0[0:NP, :] if s == 0 else tps[0:NP, off:off + cap],
                    mybir.AluOpType.add, mybir.AluOpType.mult), f"stst:{s}")
                tag_inst(nc.scalar.dma_start(out_ap[0:NP, off:off + cap],
                                             rq[:, :]), f"dma:out:{s}")

            for step in range(nslot + 2):
                # prefetch L1 weights so their ring issues precede the Pool
                # z1 ops in queue order (Pool queue is in-order; z1 waits
                # on the L0 evac and would stall the issue otherwise)
                if step + 1 < nslot:
                    wload(1, step + 1)
                if step < nslot:
                    if step not in xin:
                        load_seg(step)
                    vc_layer(step, 0, xin[step], x1)
                    xin.pop(step)
                if 1 <= step < nslot + 1:
                    vc_layer(step - 1, 1, x1[step - 1], x2)
                    x1.pop(step - 1)
                if step >= 2:
                    head_layer(step - 2)
                    x2.pop(step - 2)
                    t1b.pop(step - 2)

    nc.compile()
    import json
    with open('/tmp/imap.json', 'w') as f:
        json.dump(imap, f)
    return nc


def _prep_host(treatment, features, W0, b0, W1, b1, W2, b2):
    t32 = np.asarray(treatment, dtype=np.float32)
    t = t32.astype(np.float64)
    x = np.asarray(features, dtype=np.float32)

    order = np.argsort(t32, kind='stable')
    percore = order.reshape(N_CORES, BPC)
    kn32 = KNOTS.astype(np.float32)

    chunk_lists = []                        # per core: list of index arrays
    for c in range(N_CORES):
        gi = percore[c]
        tc_ = t32[gi]
        bounds = {0, len(gi)}
        for k in kn32:
            bounds.add(int(np.searchsorted(tc_, k, side='right')))
        bounds = sorted(bounds)
        chunks = []
        for a, b_ in zip(bounds, bounds[1:]):
            r = b_ - a
            if r <= 0:
                continue
            k = -(-r // CAPMAX)
            base, rem = divmod(r, k)
            st = a
            for i in range(k):
                n = base + (1 if i < rem else 0)
                chunks.append(gi[st:st + n])
                st += n
        chunks.sort(key=len, reverse=True)
        chunk_lists.append(chunks)

    nslot = max(len(ch) for ch in chunk_lists)
    caps = tuple(max(8, -(-max(len(ch[i]) if i < len(ch) else 0
                               for ch in chunk_lists) // 8) * 8)
                 for i in range(nslot))
    assert max(caps) <= 512, caps
    offs = np.concatenate([[0], np.cumsum(caps)]).astype(np.int64)
    bp = int(offs[-1])

    g = _gamma4()
    c4s, cb4s = [], []
    for W, b in ((W0, b0), (W1, b1)):
        Ws = np.asarray(W, dtype=np.float64).reshape(SDIM, D, D)
        c4s.append(np.einsum('mps,sio->mpio', g, Ws))
        cb4s.append(np.einsum('mps,so->mpo', g, np.asarray(b, np.float64)))
    c4h = np.einsum('mps,si->mpi', g, np.asarray(W2, np.float64))
    cb4h = np.einsum('mps,s->mp', g, np.asarray(b2, np.float64)[:, 0])

    gather = np.full((N_CORES, bp), -1, dtype=np.int64)
    in_maps = []
    for c in range(N_CORES):
        chunks = chunk_lists[c]
        xT = np.zeros((128, 2 * bp), dtype=NPBF)
        tp = np.zeros((3, bp), dtype=NPBF)
        cm = [np.zeros((nslot, 128, 768), dtype=NPBF) for _ in range(2)]
        cball = np.zeros((NP, nslot * 512), dtype=NPBF)
        c2 = np.zeros((128, nslot * 2 * NP), dtype=NPBF)
        c2b = np.zeros((NP, nslot), dtype=np.float32)
        for s, gi in enumerate(chunks):
            n, off = len(gi), int(offs[s])
            tv = t[gi]
            t0 = (tv.min() + tv.max()) / 2
            h = max((tv.max() - tv.min()) / 2, 1e-9)
            m = int(np.searchsorted(kn32, t32[gi[0]], side='right'))
            gather[c, off:off + n] = gi
            cap_s = caps[s]
            xT[:, 2 * off:2 * off + n] = x[gi, 0:128].T.astype(NPBF)
            xT[:, 2 * off + cap_s:2 * off + cap_s + n] = \
                x[gi, 128:256].T.astype(NPBF)
            dv = tv - t0
            tp[0, off:off + n] = 1.0
            tp[1, off:off + n] = dv.astype(NPBF)
            tp[2, off:off + n] = (dv * 2.0 ** -ZSH).astype(NPBF)
            for L in range(2):
                c2l = _relin(c4s[L][m], t0, h)              # (2, 256, 256)
                wc, wl8 = c2l[0], c2l[1] * 2.0 ** ZSH
                for mm in range(2):
                    for kt in range(2):
                        cm[L][s, :, (mm * 2 + kt) * 128:(mm * 2 + kt + 1) * 128] = \
                            wc[kt * 128:(kt + 1) * 128,
                               mm * 128:(mm + 1) * 128].astype(NPBF)
                cbl = _relin(cb4s[L][m], t0, h)             # (2, 256)
                cball[:, (s * 2 + L) * 256:(s * 2 + L + 1) * 256] = \
                    cbl.astype(NPBF)
                # fp8 linear weights packed as raw bytes into bf16 cols
                wl_q = np.zeros((128, 512), dtype=NPE4)
                for j in range(2):
                    wl_q[:, j * 256:(j + 1) * 256] = \
                        wl8[j * 128:(j + 1) * 128, :].astype(NPE4)
                cm[L][s, :, 512:768] = \
                    np.ascontiguousarray(wl_q).view(np.uint8).reshape(
                        128, 512).view(NPBF)
            c2h = _relin(c4h[m], t0, h)                     # (2, 256)
            for hh in range(2):
                c2[:, s * 2 * NP + hh * NP:s * 2 * NP + (hh + 1) * NP] = \
                    c2h[:, hh * 128:(hh + 1) * 128].T.astype(NPBF)
            c2b[:, s] = _relin(cb4h[m][:, None], t0, h)[:, 0].astype(np.float32)
        in_maps.append(dict(
            xT=np.ascontiguousarray(xT), tp=np.ascontiguousarray(tp),
            cm0=np.ascontiguousarray(cm[0]), cm1=np.ascontiguousarray(cm[1]),
            cball=cball, c2=c2, c2b=c2b))
    return caps, in_maps, gather


def kernel(treatment, features, W0, b0, W1, b1, W2, b2):
    global LAST_EXEC_NS, LAST_MEAN_EXEC_NS, LAST_RES
    caps, in_maps, gather = _prep_host(treatment, features, W0, b0, W1, b1,
                                       W2, b2)

    if caps not in _PROG_CACHE:
        _PROG_CACHE[caps] = _build_program(caps)
    nc = _PROG_CACHE[caps]

    if TRACE:
        _register_ntff_hook()
    res = bass_utils.run_bass_kernel_spmd(
        nc, in_maps, core_ids=list(range(N_CORES)), trace=TRACE)
    LAST_EXEC_NS = res.exec_time_ns
    LAST_MEAN_EXEC_NS = res.mean_exec_time_ns
    LAST_RES = res

    out = np.empty((B,), dtype=np.float32)
    for c in range(N_CORES):
        row = res.results[c]["out"]      # (2, bp) bf16: host sums the rows
        v = gather[c] >= 0
        s = row[0].astype(np.float32) + row[1].astype(np.float32)
        out[gather[c][v]] = s[v]
    return out.reshape(B, 1)
